# revision 24
# baseline (speedup 1.0000x reference)
"""Trainium2 Bass kernel for nn_DynaResidualBlockC (hyper-network dynamic
residual block).

Strategy (8 NeuronCores, data-parallel over batch):
  * Each core owns 2 of the 16 samples (samples 2c, 2c+1) and the full
    spatial extent for them.
  * Hypernet, two paths:
      - EXCHANGED (first): k_mid / k_out / k_short are computed shard-wise
        (each core: its 2560-row shard for ALL 16 samples) and exchanged
        with one AllToAll.  The collective pays a ~35us core-start-stagger
        penalty (the host dispatches the 8 cores sequentially), so it is
        kicked off as early as possible (~20us) and the main loop hides the
        wait behind a lag-L software pipeline.
      - LOCAL (second): every core redundantly computes k_in + all biases
        for its OWN 2 samples (lat_own @ wkin.T, skinny M=2 matmuls over a
        duplicated 4.4MB weight slice).  This unblocks the main loop's
        stage 1 with NO cross-core dependency while the collective is
        still in flight.
  * Host-side preprocessing (pure marshalling): weights pre-scaled by the
    reference's 1/sqrt(fh), 1/sqrt(fout) constants and permuted so every
    per-sample kernel lands in transposed ([in,out]) layout and every
    block-diagonal weight tile assembles with ONE flat DMA per
    (region, sample).  All inputs pre-cast to bf16.
  * Main loop, 24 macro-tiles of 1536 spatial columns (both samples packed
    on the 128 SBUF partitions).  Stage 2 lags stage 1 by LAG tiles so the
    ACT engine keeps streaming sin-waves while the exchange completes.
    PSUM: ps_in 3 banks + ps_mid 3 banks (single-buffered [128,1536] f32)
    + 2x [128,512] ps_out ping-pong = 8 banks; in the drain phase ps_mid
    ping-pongs between the psA/psB ranges (psA is free once the last
    stage-1 tile is done) so back-to-back w2 ACTs have no mid-matmul gap.
    Per macro-tile t (v = t-LAG):
        ps_in  = W_in.T @ x2                       (PE, 3x512 chunks)
        w1c/w1s = sin(ps_in + b_in (+pi/2))        (ACT, N=1536 fused bias)
        ps_mid = W_mid_c.T @ w1c + W_mid_s.T @ w1s (PE, tile v)
        w2c/w2s = sin(ps_mid + b_mid (+pi/2))      (ACT, tile v)
        ps_out = W_out_c.T@w2c + W_out_s.T@w2s + W_short.T@x2  (512 chunks)
        y      = ps_out + (b_out + b_short)        (DVE, bf16 out)
    ACT is the bottleneck engine (~148us busy); the 1536-wide ACT
    instructions amortize the 352-cycle per-instruction overhead.
  * y is stored bf16 (halves store traffic); host casts back to fp32.
"""
import ml_dtypes
import numpy as np

import concourse.bass as bass
import concourse.bacc as bacc
import concourse.mybir as mybir
from concourse import tile
from concourse.bass_utils import run_bass_kernel_spmd

# ---------------------------------------------------------------- constants
B, FIN, FOUT, FH, H2 = 16, 64, 64, 128, 64
LAT = 512
HH = WW = 192
SP = HH * WW                      # 36864 spatial positions
NCORES = 8
KIN_C = 4352                      # local wkin cols: [biases 256 | k_in 4096]
SHARD = 2560                      # exchanged shard: [k_mid 1K|k_out 1K|k_short 512]
S_TILE = 1536                     # spatial columns per main-loop macro-tile
NT = SP // S_TILE                 # 24
NCH = S_TILE // 512               # 3 x 512-col matmul chunks per tile
LAG = 10                          # stage-2 lag (rides out the A2A stagger)
PI_2 = float(np.pi / 2)

F32 = mybir.dt.float32
BF16 = mybir.dt.bfloat16
NP_BF16 = ml_dtypes.bfloat16


def _build_perms():
    """orig-row indices for the local (wkin) and sharded (wksh) layouts,
    plus the global row scales."""
    p = np.arange(KIN_C)
    orig_in = np.zeros(KIN_C, np.int64)
    # biases, col 4p+q -> bias type q element p (fills [64,4] vbias flat)
    orig_in[0:256] = 24576 + 64 * (p[0:256] % 4) + p[0:256] // 4
    t = p[256:] - 256                                # k_in.T flat
    orig_in[256:] = (t % 64) * 64 + t // 64

    orig_sh = np.zeros((NCORES, SHARD), np.int64)
    t1024 = np.arange(1024)
    t512 = np.arange(512)
    for c in range(NCORES):
        i, o = (1024 * c + t1024) // 64, (1024 * c + t1024) % 64
        orig_sh[c, 0:1024] = 4096 + o * 128 + i      # k_mid.T slice
        orig_sh[c, 1024:2048] = 12288 + o * 128 + i  # k_out.T slice
        i, o = (512 * c + t512) // 64, (512 * c + t512) % 64
        orig_sh[c, 2048:2560] = 20480 + o * 64 + i   # k_short.T slice
    s = np.ones(24832, np.float32)
    s[:12288] = 1.0 / np.sqrt(128.0)
    s[12288:24576] = 0.125
    return orig_in, orig_sh, s


def _build_nc():
    nc = bacc.Bacc(
        "TRN2",
        target_bir_lowering=False,
        debug=False,
        num_devices=NCORES,
    )
    x_d = nc.dram_tensor("x", [128, SP], BF16, kind="ExternalInput")
    # latr[p, 16q+s] = lat[s, 128q+p]; latr[0, 64:80] = 1 (16-wide ones row)
    latr_d = nc.dram_tensor("latr", [128, 80], BF16, kind="ExternalInput")
    # lato[p, 2q+u] = lat[2c+u, 128q+p]; lato[0, 8:10] = 1 (2-wide ones row)
    lato_d = nc.dram_tensor("lato", [128, 10], BF16, kind="ExternalInput")
    wkin_d = nc.dram_tensor("wkin", [LAT, KIN_C], BF16, kind="ExternalInput")
    bkin_d = nc.dram_tensor("bkin", [1, KIN_C], BF16, kind="ExternalInput")
    wksh_d = nc.dram_tensor("wksh", [LAT, SHARD], BF16, kind="ExternalInput")
    bksh_d = nc.dram_tensor("bksh", [1, SHARD], BF16, kind="ExternalInput")
    y_d = nc.dram_tensor("y", [128, SP], BF16, kind="ExternalOutput")

    SIN = mybir.ActivationFunctionType.Sin

    with tile.TileContext(nc) as tc:
        with (
            tc.tile_pool(name="wkt", bufs=1) as wkt_pool,
            tc.tile_pool(name="const", bufs=1) as const_pool,
            tc.tile_pool(name="wts", bufs=1) as w_pool,
            tc.tile_pool(name="dram", bufs=1, space="DRAM") as dram_pool,
            tc.tile_pool(name="psA", bufs=1, space=bass.MemorySpace.PSUM) as psA,
            tc.tile_pool(name="psB", bufs=1, space=bass.MemorySpace.PSUM) as psB,
            tc.tile_pool(name="psC", bufs=2, space=bass.MemorySpace.PSUM) as psC,
            tc.tile_pool(name="xin", bufs=13) as x_pool,
            tc.tile_pool(name="wv1", bufs=LAG + 1) as wave1_pool,
            tc.tile_pool(name="wv2", bufs=2) as wave2_pool,
            tc.tile_pool(name="outs", bufs=2) as out_pool,
        ):
            latr = const_pool.tile([128, 80], BF16, name="latr")
            lato = const_pool.tile([128, 10], BF16, name="lato")
            bkin = const_pool.tile([1, KIN_C], BF16, name="bkin")
            bksh = const_pool.tile([1, SHARD], BF16, name="bksh")
            lat_q = [latr[:, 16 * q:16 * q + 16] for q in range(4)]
            lato_q = [lato[:, 2 * q:2 * q + 2] for q in range(4)]
            ones16 = latr[0:1, 64:80]
            ones2 = lato[0:1, 8:10]

            # ---------- prologue DMAs: shard weights first, full-width
            # tiles (contiguous DRAM rows, no gpsimd involvement) ----------
            wks = [wkt_pool.tile([128, SHARD], BF16, name=f"ws{q}")
                   for q in range(4)]
            wki = [wkt_pool.tile([128, KIN_C], BF16, name=f"wi{q}")
                   for q in range(4)]
            nc.sync.dma_start(wks[0][:], wksh_d[0:128, :])
            nc.sync.dma_start(wks[1][:], wksh_d[128:256, :])
            nc.scalar.dma_start(wks[2][:], wksh_d[256:384, :])
            nc.scalar.dma_start(wks[3][:], wksh_d[384:512, :])
            nc.sync.dma_start(latr[:], latr_d[:])
            nc.sync.dma_start(lato[:], lato_d[:])
            nc.sync.dma_start(bksh[:], bksh_d[:])
            nc.scalar.dma_start(bkin[:], bkin_d[:])
            zscratch = const_pool.tile([128, 1], F32, name="zscratch")
            nc.scalar.activation(zscratch[:], latr[:, 0:1], SIN, bias=0.0)

            # tiny dummy matmuls release the HAM clock gate and keep it
            # open across the weight-DMA wait, so the hypernet matmuls run
            # at 2.4GHz from the start
            for _ in range(55):
                dps = psA.tile([B, B], F32, name="warm", tag="ps_in")
                nc.tensor.matmul(dps[:], latr[:, 0:16], latr[:, 16:32],
                                 start=True, stop=True)

            # main-loop weight tiles; zero them early
            W_in = w_pool.tile([128, 128], BF16, name="W_in")
            W_mid_c = w_pool.tile([128, 128], BF16, name="W_mid_c")
            W_mid_s = w_pool.tile([128, 128], BF16, name="W_mid_s")
            W_out_c = w_pool.tile([128, 128], BF16, name="W_out_c")
            W_out_s = w_pool.tile([128, 128], BF16, name="W_out_s")
            W_short = w_pool.tile([128, 128], BF16, name="W_short")
            for Wt in (W_in, W_mid_c, W_mid_s, W_out_c, W_out_s, W_short):
                nc.gpsimd.memset(Wt[:], 0.0)

            # ---------- sharded hypernet (feeds the early AllToAll) -------
            ks_sb = const_pool.tile([B, SHARD], BF16, name="ks_sb")
            n0 = 0
            while n0 < SHARD:
                nn = min(512, SHARD - n0)
                ps = psC.tile([B, nn], F32, name="hyps", tag="ps_out")
                for q in range(4):
                    nc.tensor.matmul(ps[:], lat_q[q],
                                     wks[q][:, n0:n0 + nn],
                                     start=(q == 0), stop=False)
                nc.tensor.matmul(ps[:], ones16, bksh[:, n0:n0 + nn],
                                 start=False, stop=True)
                nc.vector.tensor_copy(ks_sb[:, n0:n0 + nn], ps[:])
                n0 += nn

            # exchange: [16, 2560] sbuf rows == [8, 5120] dram rows (pairs)
            cc_in = dram_pool.tile([NCORES, 2 * SHARD], BF16, name="cc_in")
            cc_out = dram_pool.tile([NCORES, 2 * SHARD], BF16, name="cc_out")
            nc.gpsimd.dma_start(cc_in[:], ks_sb[:])
            nc.gpsimd.collective_compute(
                "AllToAll",
                mybir.AluOpType.bypass,
                replica_groups=[list(range(NCORES))],
                ins=[cc_in.opt()],
                outs=[cc_out.opt()],
            )

            # ---------- local hypernet: ks_in = lat_own @ wkin.T + bkin ---
            # (none of these DMAs may ride gpsimd: the collective dispatched
            # above blocks that queue until it completes)
            nc.sync.dma_start(wki[0][:], wkin_d[0:128, :])
            nc.sync.dma_start(wki[1][:], wkin_d[128:256, :])
            nc.scalar.dma_start(wki[2][:], wkin_d[256:384, :])
            nc.scalar.dma_start(wki[3][:], wkin_d[384:512, :])
            xts = {}
            for t in range(2):
                xt = x_pool.tile([128, S_TILE], BF16, name="xt", tag="xt")
                nc.sync.dma_start(xt[:], x_d[:, t * S_TILE:(t + 1) * S_TILE])
                xts[t] = xt

            ks_in = const_pool.tile([2, KIN_C], BF16, name="ks_in")
            n0 = 0
            while n0 < KIN_C:
                nn = min(512, KIN_C - n0)
                ps = psC.tile([2, nn], F32, name="hyps", tag="ps_out")
                for q in range(4):
                    nc.tensor.matmul(ps[:], lato_q[q],
                                     wki[q][:, n0:n0 + nn],
                                     start=(q == 0), stop=False)
                nc.tensor.matmul(ps[:], ones2, bkin[:, n0:n0 + nn],
                                 start=False, stop=True)
                nc.vector.tensor_copy(ks_in[:, n0:n0 + nn], ps[:])
                n0 += nn

            # assembly of the local path: vbias + W_in.  Must NOT ride the
            # gpsimd queue (blocked by the in-flight collective): bf16
            # reshape-DMAs on sync, then a DVE copy does the f32 cast.
            vb16 = const_pool.tile([128, 4], BF16, name="vb16")
            vbias = const_pool.tile([128, 4], F32, name="vbias")
            cvin = const_pool.tile([128, 1], F32, name="cvin")
            cvmid = const_pool.tile([128, 1], F32, name="cvmid")
            obias = const_pool.tile([128, 1], F32, name="obias")
            for smp in (0, 1):
                # bias cols are p-major so one flat DMA fills each [64, 4]
                # vb16 block per sample
                nc.sync.dma_start(
                    vb16[64 * smp:64 * smp + 64, 0:4],
                    ks_in[smp:smp + 1, 0:256],
                )
                nc.sync.dma_start(
                    W_in[64 * smp:64 * smp + 64, 64 * smp:64 * smp + 64],
                    ks_in[smp:smp + 1, 256:KIN_C],
                )
            nc.vector.tensor_copy(vbias[:], vb16[:])
            vin = vbias[:, 0:1]
            vmid = vbias[:, 1:2]
            nc.vector.tensor_scalar_add(cvin[:], vbias[:, 0:1], PI_2)
            nc.vector.tensor_scalar_add(cvmid[:], vbias[:, 1:2], PI_2)
            nc.vector.tensor_add(obias[:], vbias[:, 2:3], vbias[:, 3:4])
            for t in range(2, 4):
                # x2/x3 loads held back until here so they don't steal DMA
                # bandwidth from the wkin stream (needed only at tile 2+)
                xt = x_pool.tile([128, S_TILE], BF16, name="xt", tag="xt")
                nc.sync.dma_start(xt[:], x_d[:, t * S_TILE:(t + 1) * S_TILE])
                xts[t] = xt

            # post-exchange assembly, all on gpsimd (idle in steady state)
            for smp in (0, 1):
                base = SHARD * smp
                nc.gpsimd.dma_start(
                    W_mid_c[64 * smp:64 * smp + 64, 64 * smp:64 * smp + 64],
                    cc_out[0:4, base:base + 1024],
                )
                nc.gpsimd.dma_start(
                    W_mid_s[64 * smp:64 * smp + 64, 64 * smp:64 * smp + 64],
                    cc_out[4:8, base:base + 1024],
                )
            for smp in (0, 1):
                base = SHARD * smp
                nc.gpsimd.dma_start(
                    W_out_c[64 * smp:64 * smp + 64, 64 * smp:64 * smp + 64],
                    cc_out[0:4, base + 1024:base + 2048],
                )
                nc.gpsimd.dma_start(
                    W_out_s[64 * smp:64 * smp + 64, 64 * smp:64 * smp + 64],
                    cc_out[4:8, base + 1024:base + 2048],
                )
                nc.gpsimd.dma_start(
                    W_short[64 * smp:64 * smp + 64, 64 * smp:64 * smp + 64],
                    cc_out[:, base + 2048:base + 2560],
                )

            # ================= main loop (lag-L pipeline) =================
            # ACT order per period: w1c(t), w1s(t), w2c(v), w2s(v); v = t-LAG
            # PE order:             in(t+1), mid(v), out(v-1)
            ps_ins, ps_mids, w1_, w2_ = {}, {}, {}, {}

            ps_in = psA.tile([128, S_TILE], F32, name="ps_in", tag="ps_in")
            for ch in range(NCH):
                sl = np.s_[:, ch * 512:(ch + 1) * 512]
                nc.tensor.matmul(ps_in[sl], W_in[:], xts[0][sl],
                                 start=True, stop=True)
            ps_ins[0] = ps_in

            for t in range(NT + LAG + 2):
                v = t - LAG
                if t < NT:
                    # wave-1 ACTs for tile t
                    w1c = wave1_pool.tile([128, S_TILE], BF16, name="w1c",
                                          tag="w1c")
                    w1s = wave1_pool.tile([128, S_TILE], BF16, name="w1s",
                                          tag="w1s")
                    nc.scalar.activation(w1c[:], ps_ins[t][:], SIN,
                                         bias=cvin[:, 0:1])
                    nc.scalar.activation(w1s[:], ps_ins[t][:], SIN,
                                         bias=vin)
                    ps_ins.pop(t)
                    w1_[t] = (w1c, w1s)

                if t + 1 < NT:
                    # in-stage for tile t+1.  During the w1-only backlog
                    # phase psB is idle, so odd tiles use it as a second
                    # ps_in buffer -- the ACT w1 stream then has no
                    # in-stage bubble.  (The first mid(0) allocation takes
                    # the psB range back with a clean WAR on w1s(LAG-1).)
                    if t + 1 <= LAG - 1 and (t + 1) % 2 == 1:
                        ps_in = psB.tile([128, S_TILE], F32, name="ps_in",
                                         tag="ps_mid")
                    else:
                        ps_in = psA.tile([128, S_TILE], F32, name="ps_in",
                                         tag="ps_in")
                    for ch in range(NCH):
                        sl = np.s_[:, ch * 512:(ch + 1) * 512]
                        nc.tensor.matmul(ps_in[sl], W_in[:], xts[t + 1][sl],
                                         start=True, stop=True)
                    ps_ins[t + 1] = ps_in

                def mid_w2(v):
                    # mid-stage + wave-2 ACTs for tile v.  In the drain
                    # phase (no more in-stages) alternate ps_mid between the
                    # psB and (now free) psA bank ranges so consecutive w2
                    # ACTs have no mid-matmul bubble.
                    w1c, w1s = w1_.pop(v)
                    if v >= NT - LAG and (v - (NT - LAG)) % 2 == 1:
                        ps_mid = psA.tile([128, S_TILE], F32, name="ps_mid",
                                          tag="ps_in")
                    else:
                        ps_mid = psB.tile([128, S_TILE], F32, name="ps_mid",
                                          tag="ps_mid")
                    for ch in range(NCH):
                        sl = np.s_[:, ch * 512:(ch + 1) * 512]
                        nc.tensor.matmul(ps_mid[sl], W_mid_c[:], w1c[sl],
                                         start=True, stop=False)
                        nc.tensor.matmul(ps_mid[sl], W_mid_s[:], w1s[sl],
                                         start=False, stop=True)
                    w2c = wave2_pool.tile([128, S_TILE], BF16, name="w2c",
                                          tag="w2c")
                    w2s = wave2_pool.tile([128, S_TILE], BF16, name="w2s",
                                          tag="w2s")
                    nc.scalar.activation(w2c[:], ps_mid[:], SIN,
                                         bias=cvmid[:, 0:1])
                    nc.scalar.activation(w2s[:], ps_mid[:], SIN,
                                         bias=vmid)
                    w2_[v] = (w2c, w2s)

                def out_chunk(ot, w2c, w2s, xt_v, ch):
                    sl = np.s_[:, ch * 512:(ch + 1) * 512]
                    ps_out = psC.tile([128, 512], F32, name="ps_out",
                                      tag="ps_out")
                    nc.tensor.matmul(ps_out[:], W_out_c[:], w2c[sl],
                                     start=True, stop=False)
                    nc.tensor.matmul(ps_out[:], W_out_s[:], w2s[sl],
                                     start=False, stop=False)
                    nc.tensor.matmul(ps_out[:], W_short[:], xt_v[sl],
                                     start=False, stop=True)
                    nc.vector.tensor_scalar_add(ot[sl], ps_out[:],
                                                obias[:, 0:1])

                drain = t >= NT
                ov = v - 1
                if not drain:
                    if 0 <= v < NT:
                        mid_w2(v)
                    if 0 <= ov < NT:
                        # out-stage for tile ov, 512-col chunks through the
                        # 2-bank ps_out ping-pong, drained by DVE (bf16 out)
                        w2c, w2s = w2_.pop(ov)
                        xt_v = xts.pop(ov)
                        ot = out_pool.tile([128, S_TILE], BF16, name="ot",
                                           tag="ot")
                        for ch in range(NCH):
                            out_chunk(ot, w2c, w2s, xt_v, ch)
                        nc.sync.dma_start(
                            y_d[:, ov * S_TILE:(ov + 1) * S_TILE], ot[:])
                else:
                    # drain: sandwich mid(v) between out(ov) chunks so the
                    # PE's mid lands inside the DVE ping-pong latency and
                    # the w2 ACT stream stays gapless
                    if 0 <= ov < NT:
                        w2c, w2s = w2_.pop(ov)
                        xt_v = xts.pop(ov)
                        ot = out_pool.tile([128, S_TILE], BF16, name="ot",
                                           tag="ot")
                        out_chunk(ot, w2c, w2s, xt_v, 0)
                        out_chunk(ot, w2c, w2s, xt_v, 1)
                    if 0 <= v < NT:
                        mid_w2(v)
                    if 0 <= ov < NT:
                        out_chunk(ot, w2c, w2s, xt_v, 2)
                        nc.sync.dma_start(
                            y_d[:, ov * S_TILE:(ov + 1) * S_TILE], ot[:])

                if 4 <= t + 2 < NT:
                    # x prefetch (sync queue); 13-buffer pool covers the
                    # live range {v-1 .. t+2}; x0..x3 were pre-loaded
                    xt = x_pool.tile([128, S_TILE], BF16, name="xt", tag="xt")
                    nc.sync.dma_start(
                        xt[:], x_d[:, (t + 2) * S_TILE:(t + 3) * S_TILE])
                    xts[t + 2] = xt

    nc.compile()
    return nc


_NC_CACHE = None


def _get_nc():
    global _NC_CACHE
    if _NC_CACHE is None:
        _NC_CACHE = _build_nc()
    return _NC_CACHE


def kernel(x, lat, Wk, bk, **run_kwargs):
    x = np.asarray(x, dtype=np.float32)
    lat = np.asarray(lat, dtype=np.float32)
    Wk = np.asarray(Wk, dtype=np.float32)
    bk = np.asarray(bk, dtype=np.float32)

    orig_in, orig_sh, s = _build_perms()
    Wk_s = Wk * s[:, None]
    bk_s = bk * s
    latr = np.zeros((128, 80), np.float32)
    latr[:, 0:64] = lat.T.reshape(4, 128, B).transpose(1, 0, 2).reshape(128, 64)
    latr[0, 64:80] = 1.0
    latr_b = latr.astype(NP_BF16)
    x_b = x.reshape(B, FIN * SP).astype(NP_BF16)
    wkin = np.ascontiguousarray(Wk_s[orig_in].T.astype(NP_BF16))
    bkin = np.ascontiguousarray(bk_s[orig_in].reshape(1, KIN_C).astype(NP_BF16))

    in_maps = []
    for c in range(NCORES):
        rows = orig_sh[c]
        lato = np.zeros((128, 10), np.float32)
        lato[:, 0:8] = lat[2 * c:2 * c + 2].T.reshape(
            4, 128, 2).transpose(1, 0, 2).reshape(128, 8)
        lato[0, 8:10] = 1.0
        in_maps.append({
            "x": np.ascontiguousarray(
                x_b[2 * c:2 * c + 2].reshape(128, SP)),
            "latr": latr_b,
            "lato": lato.astype(NP_BF16),
            "wkin": wkin,
            "bkin": bkin,
            "wksh": np.ascontiguousarray(Wk_s[rows].T.astype(NP_BF16)),
            "bksh": np.ascontiguousarray(
                bk_s[rows].reshape(1, SHARD).astype(NP_BF16)),
        })

    nc = _get_nc()
    res = run_bass_kernel_spmd(nc, in_maps, core_ids=list(range(NCORES)),
                               **run_kwargs)
    y = np.empty((B, FOUT, HH, WW), np.float32)
    for c in range(NCORES):
        y[2 * c:2 * c + 2] = res.results[c]["y"].astype(
            np.float32).reshape(2, FOUT, HH, WW)
    if run_kwargs:
        kernel.last_results = res
    return y
